# revision 13
# baseline (speedup 1.0000x reference)
"""GAT layer (N=8192, D=64) as a Bass/Tile kernel on 8 TRN2 NeuronCores.

Math (reference):
    h  = x @ W.T + b
    s1 = h @ a1 ; s2 = h @ a2                    # [N] each
    score[i,j] = s2[i] + s1[j]
    att = softmax_j(leaky_relu(score))
    out = att @ x

Reformulation:
    Fold the linear layer:  v = W.T @ [a1|a2], c_k = b.a_k
      p1 = x @ v1 ; p2 = x @ v2 ; sh1 = p1 + c1 + c2
    Softmax rows are shift invariant; subtracting p2[i] from row i gives
      e[j,i] = max( E1[j], F1[j] * G2[i] )
      E1 = exp(sh1), F1 = exp(0.01*sh1), G2[i] = exp(-0.99*p2[i])
    With per-j-row scalars E1[j], F1[j] and the broadcast tile
    G2b[j,i] = G2[i], each weight tile ([j part, i free]) is ONE DVE
    tensor_scalar op:  e = max(G2b * F1[j], E1[j])  — or equivalently on
    the scalar engine  r = relu(G2b*F1[j] - E1[j]) = e - E1[j], whose
    missing rank-1 term E1[j] x ones is restored into the PSUM
    accumulators by FD=1 matmuls (q[m] = sum_j x_ext[j,m] E1[j]) plus a
    K=1 broadcast matmul.  A ones-column appended to x gives the softmax
    denominator in the same accumulation.

Sharding: each core owns N/8 = 1024 query rows i (x replicated).  The
j-tile order is rotated per core on the host so the own i-block is
tiles 0..7 — fully SPMD, own-block data arrives first.

Engine schedule (per core):
  PE    : warm-up burst (HAM at 2.4GHz before the stream), v/c prologue,
          p2 row, G2b broadcast, 128 accumulating main matmuls, q-fold,
          epilogue transposes.
  DVE   : s1 = x*v1b mul+reduce per chunk (gpsimd is AVOIDED entirely:
          its SBUF traffic knocks concurrent DVE ops out of the 4x
          perf mode), 52 fused mult+max e-tiles (critical stream).
  ACT   : exp table preload, E1/F1/G2 exps, 12 relu e-tiles, epilogue
          scaled copies.
  DMA   : sync HWDGE ring, issues at top priority, out in 4 chunks.
"""

import sys
import types

import ml_dtypes
import numpy as np

import concourse.bacc as bacc
import concourse.bass as bass
import concourse.mybir as mybir
import concourse.tile as tile
from concourse.bass_utils import run_bass_kernel_spmd


def _install_ntff_hook_shim():
    """The agent image's ``antenv`` lacks ``axon_hooks``; provide it so
    ``run_bass_kernel_spmd(trace=True)`` can capture NTFF profiles."""
    if "antenv.axon_hooks" in sys.modules:
        return
    try:
        from trn_agent_boot.trn_boot import _ntff_profile_via_ctypes

        hook = _ntff_profile_via_ctypes("/opt/axon/libaxon_pjrt.so")
        mod = types.ModuleType("antenv.axon_hooks")
        mod._hook = hook
        mod.get_axon_ntff_profile_hook = lambda: mod._hook
        mod.set_axon_ntff_profile_hook = lambda h: setattr(mod, "_hook", h)
        sys.modules["antenv.axon_hooks"] = mod
    except Exception:
        pass


_install_ntff_hook_shim()

N, D = 8192, 64
NCORES = 8
RB = N // NCORES          # rows (i) per core = 1024
NT = N // 128             # j tiles of 128 = 64
BT = RB // 128            # i tiles per core = 8
F32 = mybir.dt.float32
BF16 = mybir.dt.bfloat16
EXP = mybir.ActivationFunctionType.Exp
COPY = mybir.ActivationFunctionType.Copy
RELU = mybir.ActivationFunctionType.Relu
ADD = mybir.AluOpType.add
MUL = mybir.AluOpType.mult
MAX = mybir.AluOpType.max
AX_X = mybir.AxisListType.X
PKW = D + 131  # packed small-input width (W | b | a | ident)

# e-tiles computed on the scalar engine (relu form); the rest on DVE.
# Concentrated in early chunks so the q-fold completes mid-stream.
ACT_TILES = frozenset(
    jt for jt in range(NT) if jt % 8 in (3, 6) and jt < 48
)


def build_bass() -> bass.Bass:
    nc = bacc.Bacc(None)
    xp_d = nc.declare_dram_parameter("xp", [128, NT * D], F32, isOutput=False)
    xbf_d = nc.declare_dram_parameter(
        "xbf", [128, NT * 128], BF16, isOutput=False
    )
    xbkT_d = nc.declare_dram_parameter("xbkT", [D, RB], F32, isOutput=False)
    pk_d = nc.declare_dram_parameter("pack", [128, PKW], F32, isOutput=False)
    out_d = nc.declare_dram_parameter("out", [128, BT * D], F32, isOutput=True)

    with tile.TileContext(nc) as tc:
        with (
            tc.tile_pool(name="persist", bufs=1) as persist,
            tc.tile_pool(name="small", bufs=1) as small,
            tc.tile_pool(name="work", bufs=3) as work,
            tc.tile_pool(name="epool", bufs=12) as epool,
            tc.tile_pool(name="opool", bufs=2) as opool,
            tc.tile_pool(name="psumA", bufs=2, space="PSUM") as psumA,
            tc.tile_pool(name="psumB", bufs=1, space="PSUM") as psumB,
            tc.tile_pool(name="psumG", bufs=2, space="PSUM") as psumG,
            tc.tile_pool(name="psumR", bufs=2, space="PSUM") as psumR,
        ):
            # ---- DMA issues first, top priority, sync ring ----
            pk = small.tile([128, PKW], F32)
            xbkT_sb = small.tile([D, RB], F32)
            x_flat = persist.tile([128, NT * D], F32)
            x_sb = x_flat.rearrange("p (t d) -> p t d", t=NT)
            xbf_flat = persist.tile([128, NT * 128], BF16)
            x_bf = xbf_flat.rearrange("p (t d) -> p t d", t=NT)
            CWB = 16 * 128

            def xp_dma(c):
                nc.sync.dma_start(
                    x_flat[:, 8 * c * D : 8 * (c + 1) * D],
                    xp_d[:, 8 * c * D : 8 * (c + 1) * D],
                )

            def xbf_dma(k):
                nc.sync.dma_start(
                    xbf_flat[:, k * CWB : (k + 1) * CWB],
                    xbf_d[:, k * CWB : (k + 1) * CWB],
                )

            with tc.high_priority():
                nc.sync.dma_start(pk, pk_d[:, :])
                nc.sync.dma_start(xbkT_sb, xbkT_d[:, :])
                xp_dma(0)
                xbf_dma(0)
                xp_dma(1)
                xp_dma(2)
                xbf_dma(1)
                xp_dma(3)
                xp_dma(4)
                xbf_dma(2)
                xp_dma(5)
                xp_dma(6)
                xbf_dma(3)
                xp_dma(7)

            # ---- t~0: constants, ACT exp-table preload, PE warm-up ----
            ones_row = small.tile([1, 128], F32)
            nc.vector.memset(ones_row, 1.0)
            ones_bf = small.tile([1, 512], BF16)
            nc.vector.memset(ones_bf, 1.0)
            dummy_e = small.tile([1, 1], F32)
            nc.scalar.activation(out=dummy_e, in_=ones_row[0:1, 0:1], func=EXP)
            junk_ps = psumG.tile([128, 512], F32, tag="gb", name="junk_ps")
            for w in range(5):
                nc.tensor.matmul(
                    junk_ps,
                    lhsT=ones_bf[0:1, 0:128],
                    rhs=ones_bf,
                    start=True,
                    stop=True,
                )

            # ---------------- prologue + G2b (high priority) --------------
            # PE order avoids stalling on ACT round-trips: both p2 rows
            # first, then v1b/c12 (needed by the s1 pipeline), then the
            # G2b broadcasts once the exps are back.
            ident = pk[:, D + 3 : D + 3 + 128]
            with tc.high_priority():
                v_ps = psumA.tile([D, 2], F32, tag="ps", name="v_ps")
                nc.tensor.matmul(
                    v_ps,
                    lhsT=pk[0:D, 0:D],
                    rhs=pk[0:D, D + 1 : D + 3],
                    start=True,
                    stop=True,
                )
                v_sb = small.tile([D, 2], F32)
                nc.vector.tensor_copy(v_sb, v_ps)

                g2row = small.tile([1, RB], BF16)
                G2b = persist.tile([128, RB], BF16)
                p2r_pss = []
                for h in range(2):
                    p2r_ps = psumR.tile(
                        [1, 512], F32, tag="p2r", name="p2r_ps"
                    )
                    nc.tensor.matmul(
                        p2r_ps,
                        lhsT=v_sb[:, 1:2],
                        rhs=xbkT_sb[:, h * 512 : (h + 1) * 512],
                        start=True,
                        stop=True,
                    )
                    nc.scalar.activation(
                        out=g2row[:, h * 512 : (h + 1) * 512],
                        in_=p2r_ps,
                        func=EXP,
                        scale=-0.99,
                    )
                    p2r_pss.append(p2r_ps)

                # v1 broadcast + c12 while ACT computes the g2 rows
                vr_ps = psumA.tile([2, D], F32, tag="ps", name="vr_ps")
                nc.tensor.transpose(vr_ps, v_sb, ident[:D, :D])
                vrow = small.tile([2, D], F32)
                nc.vector.tensor_copy(vrow, vr_ps)
                v1b_ps = psumA.tile([128, D], F32, tag="ps", name="v1b_ps")
                nc.tensor.matmul(
                    v1b_ps, lhsT=ones_row, rhs=vrow[0:1, :], start=True, stop=True
                )
                v1b = small.tile([128, D], F32)
                nc.vector.tensor_copy(v1b, v1b_ps)
                c_ps = psumA.tile([1, 2], F32, tag="ps", name="c_ps")
                nc.tensor.matmul(
                    c_ps,
                    lhsT=pk[0:D, D : D + 1],
                    rhs=pk[0:D, D + 1 : D + 3],
                    start=True,
                    stop=True,
                )
                c_sb = small.tile([1, 2], F32)
                nc.vector.tensor_copy(c_sb, c_ps)
                cb_ps = psumA.tile([128, 2], F32, tag="ps", name="cb_ps")
                nc.tensor.matmul(
                    cb_ps, lhsT=ones_row, rhs=c_sb, start=True, stop=True
                )
                c12 = small.tile([128, 1], F32)
                nc.vector.tensor_reduce(out=c12, in_=cb_ps, axis=AX_X, op=ADD)
                c12s = small.tile([128, 1], F32)
                nc.vector.tensor_scalar(
                    out=c12s, in0=c12, scalar1=0.01, scalar2=None, op0=MUL
                )

                for h in range(2):
                    gb_ps = psumG.tile([128, 512], F32, tag="gb", name="gb_ps")
                    nc.tensor.matmul(
                        gb_ps,
                        lhsT=ones_bf[0:1, 0:128],
                        rhs=g2row[:, h * 512 : (h + 1) * 512],
                        start=True,
                        stop=True,
                    )
                    if h == 0:
                        nc.scalar.copy(out=G2b[:, 0:512], in_=gb_ps)
                    else:
                        nc.vector.tensor_copy(G2b[:, 512:1024], gb_ps)

            v1b_b = bass.AP(
                tensor=v1b.tensor,
                offset=v1b.offset,
                ap=[v1b.ap[0], [0, 8], v1b.ap[1]],
            )

            # ---------------- s1 pipeline (DVE mul + reduce) ---------------
            s1c = small.tile([128, NT], F32)
            E1c = small.tile([128, NT], F32)
            nE1c = small.tile([128, NT], F32)   # -E1 (relu-form bias)
            E1bf = small.tile([128, NT], BF16)  # E1 (q-matmul rhs)
            F1c = small.tile([128, NT], F32)

            def s1_chunk(c):
                sl = slice(8 * c, 8 * (c + 1))
                tmp = work.tile([128, 8, D], F32, tag="tmp", name="tmp")
                nc.vector.tensor_mul(tmp, x_sb[:, sl, :], v1b_b)
                nc.vector.tensor_reduce(
                    out=s1c[:, sl], in_=tmp, axis=AX_X, op=ADD
                )
                nc.scalar.activation(
                    out=E1c[:, sl], in_=s1c[:, sl], func=EXP, bias=c12,
                    scale=1.0,
                )
                nc.scalar.activation(
                    out=F1c[:, sl], in_=s1c[:, sl], func=EXP, bias=c12s,
                    scale=0.01,
                )
                nc.vector.tensor_scalar(
                    out=nE1c[:, sl], in0=E1c[:, sl], scalar1=-1.0,
                    scalar2=None, op0=MUL,
                )
                nc.vector.tensor_copy(E1bf[:, sl], E1c[:, sl])

            for c in range(3):
                s1_chunk(c)

            # ---------------- main stream ----------------
            acc0 = psumB.tile([128, 512], F32, tag="acc0", name="acc0")
            acc1 = psumB.tile([128, 512], F32, tag="acc1", name="acc1")
            accs = [acc0, acc1]
            q_ps = psumG.tile([128, 1], F32, tag="gb", name="q_ps")
            n_act = 0
            for jt in range(NT):
                if jt % 8 == 0 and jt // 8 + 3 < 8:
                    s1_chunk(jt // 8 + 3)
                if jt == 49:
                    # fold q back mid-stream: acc[m,i] += q[m] (K=1 matmul)
                    q_sb = small.tile([128, 1], F32)
                    nc.vector.tensor_copy(q_sb, q_ps)
                    qr_ps = psumR.tile([1, 128], F32, tag="p2r", name="qr_ps")
                    nc.tensor.transpose(qr_ps, q_sb, ident[:128, :128])
                    qrow = small.tile([1, 128], BF16)
                    nc.vector.tensor_copy(qrow, qr_ps)
                    for h in range(2):
                        nc.tensor.matmul(
                            accs[h],
                            lhsT=qrow,
                            rhs=ones_bf,
                            start=False,
                            stop=False,
                        )
                e_t = epool.tile([128, RB], BF16, tag="e", name="e_t")
                if jt in ACT_TILES:
                    # r = relu(G2b*F1[j] - E1[j]) on the scalar engine
                    nc.scalar.activation(
                        out=e_t,
                        in_=G2b,
                        func=RELU,
                        scale=F1c[:, jt : jt + 1],
                        bias=nE1c[:, jt : jt + 1],
                    )
                else:
                    # e = max(G2b*F1[j], E1[j]) on DVE
                    nc.vector.tensor_scalar(
                        out=e_t,
                        in0=G2b,
                        scalar1=F1c[:, jt : jt + 1],
                        scalar2=E1c[:, jt : jt + 1],
                        op0=MUL,
                        op1=MAX,
                    )
                for h in range(2):
                    nc.tensor.matmul(
                        accs[h],
                        lhsT=x_bf[:, jt, 0:128],
                        rhs=e_t[:, h * 512 : (h + 1) * 512],
                        start=(jt == 0),
                        stop=(jt == NT - 1),
                    )
                if jt in ACT_TILES:
                    # q += x_ext[j,:].T @ E1[j]  (the subtracted rank-1 term)
                    n_act += 1
                    nc.tensor.matmul(
                        q_ps,
                        lhsT=x_bf[:, jt, 0:128],
                        rhs=E1bf[:, jt : jt + 1],
                        start=(n_act == 1),
                        stop=(n_act == len(ACT_TILES)),
                    )

            # ---------------- epilogue: normalize + store ----------------
            outT = small.tile([D + 1, RB], F32)
            nc.vector.tensor_copy(outT[:, 0:256], acc0[0 : D + 1, 0:256])
            nc.scalar.copy(out=outT[:, 512:768], in_=acc1[0 : D + 1, 0:256])
            nc.vector.tensor_copy(outT[:, 256:512], acc0[0 : D + 1, 256:512])
            nc.scalar.copy(out=outT[:, 768:1024], in_=acc1[0 : D + 1, 256:512])
            out_flat = small.tile([128, BT * D], F32)
            out_sb = out_flat.rearrange("p (t d) -> p t d", t=BT)
            tp_pools = [(psumA, "ps"), (psumR, "p2r"), (psumG, "gb")]
            for t in range(BT):
                pool, tag = tp_pools[t % 3]
                tp2 = pool.tile([128, D + 1], F32, tag=tag, name="tp2")
                nc.tensor.transpose(
                    tp2, outT[:, t * 128 : (t + 1) * 128], ident[: D + 1, : D + 1]
                )
                rcol = opool.tile([128, 1], F32, tag="rcol", name="rcol")
                nc.vector.reciprocal(rcol, tp2[:, D : D + 1])
                nc.scalar.activation(
                    out=out_sb[:, t, :],
                    in_=tp2[:, 0:D],
                    func=COPY,
                    scale=rcol,
                )
                if t % 2 == 1:
                    nc.sync.dma_start(
                        out_d[:, (t - 1) * D : (t + 1) * D],
                        out_flat[:, (t - 1) * D : (t + 1) * D],
                    )

    nc.finalize()
    return nc


def _execute(inputs: dict, trace: bool = False):
    x = np.ascontiguousarray(np.asarray(inputs["x"], dtype=np.float32))
    W = np.ascontiguousarray(np.asarray(inputs["W"], dtype=np.float32))
    b = np.ascontiguousarray(
        np.asarray(inputs["b"], dtype=np.float32).reshape(D, 1)
    )
    a = np.ascontiguousarray(
        np.asarray(inputs["a"], dtype=np.float32).reshape(2 * D, 1)
    )
    assert x.shape == (N, D) and W.shape == (D, D)

    nc = build_bass()
    pack0 = np.zeros((128, PKW), np.float32)
    pack0[0:D, 0:D] = W
    pack0[0:D, D] = b[:, 0]
    pack0[0:D, D + 1] = a[:D, 0]
    pack0[0:D, D + 2] = a[D:, 0]
    pack0[:, D + 3 : D + 131] = np.eye(128, dtype=np.float32)

    # host-side layout only: partition-major f32 x, bf16 x_ext, d-major
    # own-block x.T; j-tile order rotated per core (own block first)
    xp_tiles = x.reshape(NT, 128, D).transpose(1, 0, 2)  # [128, NT, D]
    xe = np.concatenate(
        [x, np.ones((N, 1), np.float32), np.zeros((N, 127 - D), np.float32)],
        axis=1,
    )
    xbf_tiles = (
        xe.reshape(NT, 128, 128).transpose(1, 0, 2).astype(ml_dtypes.bfloat16)
    )  # [128, NT, 128]
    in_maps = []
    for c in range(NCORES):
        rot = np.roll(np.arange(NT), -c * BT)  # own tiles first
        xp_c = np.ascontiguousarray(xp_tiles[:, rot, :].reshape(128, NT * D))
        xbf_c = np.ascontiguousarray(
            xbf_tiles[:, rot, :].reshape(128, NT * 128)
        )
        xbkT = np.ascontiguousarray(x[c * RB : (c + 1) * RB].T)
        in_maps.append(
            {"xp": xp_c, "xbf": xbf_c, "xbkT": xbkT, "pack": pack0}
        )
    res = run_bass_kernel_spmd(
        nc, in_maps, core_ids=list(range(NCORES)), trace=trace
    )
    # un-permute each core's output: (p, t*D+d) -> (t*128+p, d)
    outs = []
    for r in res.results:
        o = r["out"].reshape(128, BT, D).transpose(1, 0, 2).reshape(RB, D)
        outs.append(o)
    out = np.ascontiguousarray(np.concatenate(outs, axis=0))
    return out, res


def kernel(x, W, b, a):
    out, _ = _execute({"x": x, "W": W, "b": b, "a": a})
    return out


# revision 14
# speedup vs baseline: 1.0144x; 1.0144x over previous
"""GAT layer (N=8192, D=64) as a Bass/Tile kernel on 8 TRN2 NeuronCores.

Math (reference):
    h  = x @ W.T + b
    s1 = h @ a1 ; s2 = h @ a2                    # [N] each
    score[i,j] = s2[i] + s1[j]
    att = softmax_j(leaky_relu(score))
    out = att @ x

Reformulation:
    Fold the linear layer:  v = W.T @ [a1|a2], c_k = b.a_k
      p1 = x @ v1 ; p2 = x @ v2 ; sh1 = p1 + c1 + c2
    Softmax rows are shift invariant; subtracting p2[i] from row i gives
      e[j,i] = max( E1[j], F1[j] * G2[i] )
      E1 = exp(sh1), F1 = exp(0.01*sh1), G2[i] = exp(-0.99*p2[i])
    With per-j-row scalars E1[j], F1[j] and the broadcast tile
    G2b[j,i] = G2[i], each weight tile ([j part, i free]) is ONE DVE
    tensor_scalar op:  e = max(G2b * F1[j], E1[j])  — or equivalently on
    the scalar engine  r = relu(G2b*F1[j] - E1[j]) = e - E1[j], whose
    missing rank-1 term E1[j] x ones is restored into the PSUM
    accumulators by FD=1 matmuls (q[m] = sum_j x_ext[j,m] E1[j]) plus a
    K=1 broadcast matmul.  A ones-column appended to x gives the softmax
    denominator in the same accumulation.

Sharding: each core owns N/8 = 1024 query rows i (x replicated).  The
j-tile order is rotated per core on the host so the own i-block is
tiles 0..7 — fully SPMD, own-block data arrives first.

Engine schedule (per core):
  PE    : warm-up burst (HAM at 2.4GHz before the stream), v/c prologue,
          p2 row, G2b broadcast, 128 accumulating main matmuls, q-fold,
          epilogue transposes.
  DVE   : s1 = x*v1b mul+reduce per chunk (gpsimd is AVOIDED entirely:
          its SBUF traffic knocks concurrent DVE ops out of the 4x
          perf mode), 52 fused mult+max e-tiles (critical stream).
  ACT   : exp table preload, E1/F1/G2 exps, 12 relu e-tiles, epilogue
          scaled copies.
  DMA   : sync HWDGE ring, issues at top priority, out in 4 chunks.
"""

import sys
import types

import ml_dtypes
import numpy as np

import concourse.bacc as bacc
import concourse.bass as bass
import concourse.mybir as mybir
import concourse.tile as tile
from concourse.bass_utils import run_bass_kernel_spmd


def _install_ntff_hook_shim():
    """The agent image's ``antenv`` lacks ``axon_hooks``; provide it so
    ``run_bass_kernel_spmd(trace=True)`` can capture NTFF profiles."""
    if "antenv.axon_hooks" in sys.modules:
        return
    try:
        from trn_agent_boot.trn_boot import _ntff_profile_via_ctypes

        hook = _ntff_profile_via_ctypes("/opt/axon/libaxon_pjrt.so")
        mod = types.ModuleType("antenv.axon_hooks")
        mod._hook = hook
        mod.get_axon_ntff_profile_hook = lambda: mod._hook
        mod.set_axon_ntff_profile_hook = lambda h: setattr(mod, "_hook", h)
        sys.modules["antenv.axon_hooks"] = mod
    except Exception:
        pass


_install_ntff_hook_shim()

N, D = 8192, 64
NCORES = 8
RB = N // NCORES          # rows (i) per core = 1024
NT = N // 128             # j tiles of 128 = 64
BT = RB // 128            # i tiles per core = 8
F32 = mybir.dt.float32
BF16 = mybir.dt.bfloat16
EXP = mybir.ActivationFunctionType.Exp
COPY = mybir.ActivationFunctionType.Copy
RELU = mybir.ActivationFunctionType.Relu
ADD = mybir.AluOpType.add
MUL = mybir.AluOpType.mult
MAX = mybir.AluOpType.max
AX_X = mybir.AxisListType.X
PKW = D + 131  # packed small-input width (W | b | a | ident)

# e-tiles computed on the scalar engine (relu form); the rest on DVE.
# Concentrated in early chunks so the q-fold completes mid-stream.
ACT_TILES = frozenset(
    jt for jt in range(NT) if jt % 8 in (3, 6) and jt < 48
)


def build_bass() -> bass.Bass:
    nc = bacc.Bacc(None)
    xp_d = nc.declare_dram_parameter("xp", [128, NT * D], F32, isOutput=False)
    xbf_d = nc.declare_dram_parameter(
        "xbf", [128, NT * 128], BF16, isOutput=False
    )
    xbkT_d = nc.declare_dram_parameter("xbkT", [D, RB], F32, isOutput=False)
    pk_d = nc.declare_dram_parameter("pack", [128, PKW], F32, isOutput=False)
    out_d = nc.declare_dram_parameter("out", [128, BT * D], F32, isOutput=True)

    with tile.TileContext(nc) as tc:
        with (
            tc.tile_pool(name="persist", bufs=1) as persist,
            tc.tile_pool(name="small", bufs=1) as small,
            tc.tile_pool(name="work", bufs=3) as work,
            tc.tile_pool(name="epool", bufs=12) as epool,
            tc.tile_pool(name="opool", bufs=2) as opool,
            tc.tile_pool(name="psumA", bufs=2, space="PSUM") as psumA,
            tc.tile_pool(name="psumB", bufs=1, space="PSUM") as psumB,
            tc.tile_pool(name="psumG", bufs=2, space="PSUM") as psumG,
            tc.tile_pool(name="psumR", bufs=2, space="PSUM") as psumR,
        ):
            # ---- DMA issues first, top priority, sync ring ----
            pk = small.tile([128, PKW], F32)
            xbkT_sb = small.tile([D, RB], F32)
            x_flat = persist.tile([128, NT * D], F32)
            x_sb = x_flat.rearrange("p (t d) -> p t d", t=NT)
            xbf_flat = persist.tile([128, NT * 128], BF16)
            x_bf = xbf_flat.rearrange("p (t d) -> p t d", t=NT)
            CWB = 16 * 128

            def xp_dma(c):
                nc.sync.dma_start(
                    x_flat[:, 8 * c * D : 8 * (c + 1) * D],
                    xp_d[:, 8 * c * D : 8 * (c + 1) * D],
                )

            def xbf_dma(k):
                nc.sync.dma_start(
                    xbf_flat[:, k * CWB : (k + 1) * CWB],
                    xbf_d[:, k * CWB : (k + 1) * CWB],
                )

            with tc.high_priority():
                nc.sync.dma_start(pk, pk_d[:, :])
                nc.sync.dma_start(xbkT_sb, xbkT_d[:, :])
                xp_dma(0)
                xbf_dma(0)
                xp_dma(1)
                xp_dma(2)
                xbf_dma(1)
                xp_dma(3)
                xp_dma(4)
                xbf_dma(2)
                xp_dma(5)
                xp_dma(6)
                xbf_dma(3)
                xp_dma(7)

            # ---- t~0: constants, ACT exp-table preload, PE warm-up ----
            ones_row = small.tile([1, 128], F32)
            nc.vector.memset(ones_row, 1.0)
            ones_bf = small.tile([1, 512], BF16)
            nc.vector.memset(ones_bf, 1.0)
            dummy_e = small.tile([1, 1], F32)
            nc.scalar.activation(out=dummy_e, in_=ones_row[0:1, 0:1], func=EXP)
            junk_ps = psumG.tile([128, 512], F32, tag="gb", name="junk_ps")
            for w in range(5):
                nc.tensor.matmul(
                    junk_ps,
                    lhsT=ones_bf[0:1, 0:128],
                    rhs=ones_bf,
                    start=True,
                    stop=True,
                )

            # ---------------- prologue + G2b (high priority) --------------
            # PE order avoids stalling on ACT round-trips: both p2 rows
            # first, then v1b/c12 (needed by the s1 pipeline), then the
            # G2b broadcasts once the exps are back.
            ident = pk[:, D + 3 : D + 3 + 128]
            with tc.high_priority():
                v_ps = psumA.tile([D, 2], F32, tag="ps", name="v_ps")
                nc.tensor.matmul(
                    v_ps,
                    lhsT=pk[0:D, 0:D],
                    rhs=pk[0:D, D + 1 : D + 3],
                    start=True,
                    stop=True,
                )
                v_sb = small.tile([D, 2], F32)
                nc.vector.tensor_copy(v_sb, v_ps)

                g2row = small.tile([1, RB], BF16)
                G2b = persist.tile([128, RB], BF16)
                p2r_pss = []
                for h in range(2):
                    p2r_ps = psumR.tile(
                        [1, 512], F32, tag="p2r", name="p2r_ps"
                    )
                    nc.tensor.matmul(
                        p2r_ps,
                        lhsT=v_sb[:, 1:2],
                        rhs=xbkT_sb[:, h * 512 : (h + 1) * 512],
                        start=True,
                        stop=True,
                    )
                    nc.scalar.activation(
                        out=g2row[:, h * 512 : (h + 1) * 512],
                        in_=p2r_ps,
                        func=EXP,
                        scale=-0.99,
                    )
                    p2r_pss.append(p2r_ps)

                # v1 broadcast + c12 while ACT computes the g2 rows
                vr_ps = psumA.tile([2, D], F32, tag="ps", name="vr_ps")
                nc.tensor.transpose(vr_ps, v_sb, ident[:D, :D])
                vrow = small.tile([2, D], F32)
                nc.vector.tensor_copy(vrow, vr_ps)
                v1b_ps = psumA.tile([128, D], F32, tag="ps", name="v1b_ps")
                nc.tensor.matmul(
                    v1b_ps, lhsT=ones_row, rhs=vrow[0:1, :], start=True, stop=True
                )
                v1b = small.tile([128, D], F32)
                nc.vector.tensor_copy(v1b, v1b_ps)
                c_ps = psumA.tile([1, 2], F32, tag="ps", name="c_ps")
                nc.tensor.matmul(
                    c_ps,
                    lhsT=pk[0:D, D : D + 1],
                    rhs=pk[0:D, D + 1 : D + 3],
                    start=True,
                    stop=True,
                )
                c_sb = small.tile([1, 2], F32)
                nc.vector.tensor_copy(c_sb, c_ps)
                cb_ps = psumA.tile([128, 2], F32, tag="ps", name="cb_ps")
                nc.tensor.matmul(
                    cb_ps, lhsT=ones_row, rhs=c_sb, start=True, stop=True
                )
                c12 = small.tile([128, 1], F32)
                nc.vector.tensor_reduce(out=c12, in_=cb_ps, axis=AX_X, op=ADD)
                c12s = small.tile([128, 1], F32)
                nc.vector.tensor_scalar(
                    out=c12s, in0=c12, scalar1=0.01, scalar2=None, op0=MUL
                )

                for h in range(2):
                    gb_ps = psumG.tile([128, 512], F32, tag="gb", name="gb_ps")
                    nc.tensor.matmul(
                        gb_ps,
                        lhsT=ones_bf[0:1, 0:128],
                        rhs=g2row[:, h * 512 : (h + 1) * 512],
                        start=True,
                        stop=True,
                    )
                    if h == 0:
                        nc.scalar.copy(out=G2b[:, 0:512], in_=gb_ps)
                    else:
                        nc.vector.tensor_copy(G2b[:, 512:1024], gb_ps)

            v1b_b = bass.AP(
                tensor=v1b.tensor,
                offset=v1b.offset,
                ap=[v1b.ap[0], [0, 8], v1b.ap[1]],
            )

            # ---------------- s1 pipeline (DVE mul + reduce) ---------------
            s1c = small.tile([128, NT], F32)
            E1c = small.tile([128, NT], F32)
            nE1c = small.tile([128, NT], F32)   # -E1 (relu-form bias)
            E1bf = small.tile([128, NT], BF16)  # E1 (q-matmul rhs)
            F1c = small.tile([128, NT], F32)

            def s1_chunk(c):
                sl = slice(8 * c, 8 * (c + 1))
                tmp = work.tile([128, 8, D], F32, tag="tmp", name="tmp")
                nc.vector.tensor_mul(tmp, x_sb[:, sl, :], v1b_b)
                nc.vector.tensor_reduce(
                    out=s1c[:, sl], in_=tmp, axis=AX_X, op=ADD
                )
                nc.scalar.activation(
                    out=E1c[:, sl], in_=s1c[:, sl], func=EXP, bias=c12,
                    scale=1.0,
                )
                nc.scalar.activation(
                    out=F1c[:, sl], in_=s1c[:, sl], func=EXP, bias=c12s,
                    scale=0.01,
                )
                nc.vector.tensor_scalar(
                    out=nE1c[:, sl], in0=E1c[:, sl], scalar1=-1.0,
                    scalar2=None, op0=MUL,
                )
                nc.vector.tensor_copy(E1bf[:, sl], E1c[:, sl])

            for c in range(4):
                s1_chunk(c)

            # ---------------- main stream ----------------
            acc0 = psumB.tile([128, 512], F32, tag="acc0", name="acc0")
            acc1 = psumB.tile([128, 512], F32, tag="acc1", name="acc1")
            accs = [acc0, acc1]
            q_ps = psumG.tile([128, 1], F32, tag="gb", name="q_ps")
            n_act = 0
            for jt in range(NT):
                if jt % 8 == 4 and jt // 8 + 4 < 8:
                    s1_chunk(jt // 8 + 4)
                if jt == 49:
                    # fold q back mid-stream: acc[m,i] += q[m] (K=1 matmul)
                    q_sb = small.tile([128, 1], F32)
                    nc.vector.tensor_copy(q_sb, q_ps)
                    qr_ps = psumR.tile([1, 128], F32, tag="p2r", name="qr_ps")
                    nc.tensor.transpose(qr_ps, q_sb, ident[:128, :128])
                    qrow = small.tile([1, 128], BF16)
                    nc.vector.tensor_copy(qrow, qr_ps)
                    for h in range(2):
                        nc.tensor.matmul(
                            accs[h],
                            lhsT=qrow,
                            rhs=ones_bf,
                            start=False,
                            stop=False,
                        )
                e_t = epool.tile([128, RB], BF16, tag="e", name="e_t")
                if jt in ACT_TILES:
                    # r = relu(G2b*F1[j] - E1[j]) on the scalar engine
                    nc.scalar.activation(
                        out=e_t,
                        in_=G2b,
                        func=RELU,
                        scale=F1c[:, jt : jt + 1],
                        bias=nE1c[:, jt : jt + 1],
                    )
                else:
                    # e = max(G2b*F1[j], E1[j]) on DVE
                    nc.vector.tensor_scalar(
                        out=e_t,
                        in0=G2b,
                        scalar1=F1c[:, jt : jt + 1],
                        scalar2=E1c[:, jt : jt + 1],
                        op0=MUL,
                        op1=MAX,
                    )
                for h in range(2):
                    nc.tensor.matmul(
                        accs[h],
                        lhsT=x_bf[:, jt, 0:128],
                        rhs=e_t[:, h * 512 : (h + 1) * 512],
                        start=(jt == 0),
                        stop=(jt == NT - 1),
                    )
                if jt in ACT_TILES:
                    # q += x_ext[j,:].T @ E1[j]  (the subtracted rank-1 term)
                    n_act += 1
                    nc.tensor.matmul(
                        q_ps,
                        lhsT=x_bf[:, jt, 0:128],
                        rhs=E1bf[:, jt : jt + 1],
                        start=(n_act == 1),
                        stop=(n_act == len(ACT_TILES)),
                    )

            # ---------------- epilogue: normalize + store ----------------
            outT = small.tile([D + 1, RB], F32)
            nc.vector.tensor_copy(outT[:, 0:256], acc0[0 : D + 1, 0:256])
            nc.scalar.copy(out=outT[:, 512:768], in_=acc1[0 : D + 1, 0:256])
            nc.vector.tensor_copy(outT[:, 256:512], acc0[0 : D + 1, 256:512])
            nc.scalar.copy(out=outT[:, 768:1024], in_=acc1[0 : D + 1, 256:512])
            out_flat = small.tile([128, BT * D], F32)
            out_sb = out_flat.rearrange("p (t d) -> p t d", t=BT)
            tp_pools = [(psumA, "ps"), (psumR, "p2r"), (psumG, "gb")]
            for t in range(BT):
                pool, tag = tp_pools[t % 3]
                tp2 = pool.tile([128, D + 1], F32, tag=tag, name="tp2")
                nc.tensor.transpose(
                    tp2, outT[:, t * 128 : (t + 1) * 128], ident[: D + 1, : D + 1]
                )
                rcol = opool.tile([128, 1], F32, tag="rcol", name="rcol")
                nc.vector.reciprocal(rcol, tp2[:, D : D + 1])
                nc.scalar.activation(
                    out=out_sb[:, t, :],
                    in_=tp2[:, 0:D],
                    func=COPY,
                    scale=rcol,
                )
                if t % 2 == 1:
                    nc.sync.dma_start(
                        out_d[:, (t - 1) * D : (t + 1) * D],
                        out_flat[:, (t - 1) * D : (t + 1) * D],
                    )

    nc.finalize()
    return nc


def _execute(inputs: dict, trace: bool = False):
    x = np.ascontiguousarray(np.asarray(inputs["x"], dtype=np.float32))
    W = np.ascontiguousarray(np.asarray(inputs["W"], dtype=np.float32))
    b = np.ascontiguousarray(
        np.asarray(inputs["b"], dtype=np.float32).reshape(D, 1)
    )
    a = np.ascontiguousarray(
        np.asarray(inputs["a"], dtype=np.float32).reshape(2 * D, 1)
    )
    assert x.shape == (N, D) and W.shape == (D, D)

    nc = build_bass()
    pack0 = np.zeros((128, PKW), np.float32)
    pack0[0:D, 0:D] = W
    pack0[0:D, D] = b[:, 0]
    pack0[0:D, D + 1] = a[:D, 0]
    pack0[0:D, D + 2] = a[D:, 0]
    pack0[:, D + 3 : D + 131] = np.eye(128, dtype=np.float32)

    # host-side layout only: partition-major f32 x, bf16 x_ext, d-major
    # own-block x.T; j-tile order rotated per core (own block first)
    xp_tiles = x.reshape(NT, 128, D).transpose(1, 0, 2)  # [128, NT, D]
    xe = np.concatenate(
        [x, np.ones((N, 1), np.float32), np.zeros((N, 127 - D), np.float32)],
        axis=1,
    )
    xbf_tiles = (
        xe.reshape(NT, 128, 128).transpose(1, 0, 2).astype(ml_dtypes.bfloat16)
    )  # [128, NT, 128]
    in_maps = []
    for c in range(NCORES):
        rot = np.roll(np.arange(NT), -c * BT)  # own tiles first
        xp_c = np.ascontiguousarray(xp_tiles[:, rot, :].reshape(128, NT * D))
        xbf_c = np.ascontiguousarray(
            xbf_tiles[:, rot, :].reshape(128, NT * 128)
        )
        xbkT = np.ascontiguousarray(x[c * RB : (c + 1) * RB].T)
        in_maps.append(
            {"xp": xp_c, "xbf": xbf_c, "xbkT": xbkT, "pack": pack0}
        )
    res = run_bass_kernel_spmd(
        nc, in_maps, core_ids=list(range(NCORES)), trace=trace
    )
    # un-permute each core's output: (p, t*D+d) -> (t*128+p, d)
    outs = []
    for r in res.results:
        o = r["out"].reshape(128, BT, D).transpose(1, 0, 2).reshape(RB, D)
        outs.append(o)
    out = np.ascontiguousarray(np.concatenate(outs, axis=0))
    return out, res


def kernel(x, W, b, a):
    out, _ = _execute({"x": x, "W": W, "b": b, "a": a})
    return out


# revision 15
# speedup vs baseline: 1.0199x; 1.0054x over previous
"""GAT layer (N=8192, D=64) as a Bass/Tile kernel on 8 TRN2 NeuronCores.

Math (reference):
    h  = x @ W.T + b
    s1 = h @ a1 ; s2 = h @ a2                    # [N] each
    score[i,j] = s2[i] + s1[j]
    att = softmax_j(leaky_relu(score))
    out = att @ x

Reformulation:
    Fold the linear layer:  v = W.T @ [a1|a2], c_k = b.a_k
      p1 = x @ v1 ; p2 = x @ v2 ; sh1 = p1 + c1 + c2
    Softmax rows are shift invariant; subtracting p2[i] from row i gives
      e[j,i] = max( E1[j], F1[j] * G2[i] )
      E1 = exp(sh1), F1 = exp(0.01*sh1), G2[i] = exp(-0.99*p2[i])
    With per-j-row scalars E1[j], F1[j] and the broadcast tile
    G2b[j,i] = G2[i], each weight tile ([j part, i free]) is ONE DVE
    tensor_scalar op:  e = max(G2b * F1[j], E1[j])  — or equivalently on
    the scalar engine  r = relu(G2b*F1[j] - E1[j]) = e - E1[j], whose
    missing rank-1 term E1[j] x ones is restored into the PSUM
    accumulators by FD=1 matmuls (q[m] = sum_j x_ext[j,m] E1[j]) plus a
    K=1 broadcast matmul mid-stream.  A ones-column appended to x gives
    the softmax denominator in the same accumulation.

Sharding: each core owns N/8 = 1024 query rows i (x replicated).  The
j-tile order is rotated per core on the host so the own i-block is
tiles 0..7 — fully SPMD, own-block data arrives first.

Engine notes:
  - gpsimd is left idle ON PURPOSE: its SBUF traffic knocks concurrent
    DVE ops from the 4x perf mode (456ns/tile) down to 2x/1x (686-1400).
  - PE filler matmuls are interleaved into the first j-tiles: the HAM
    clock gate needs ~3.4us of sustained PE activity to unthrottle
    1.2 -> 2.4 GHz, and the stream ramp has starvation gaps that would
    otherwise keep the PE cold (and trailing) until ~27us.
"""

import sys
import types

import ml_dtypes
import numpy as np

import concourse.bacc as bacc
import concourse.bass as bass
import concourse.mybir as mybir
import concourse.tile as tile
from concourse.bass_utils import run_bass_kernel_spmd


def _install_ntff_hook_shim():
    """The agent image's ``antenv`` lacks ``axon_hooks``; provide it so
    ``run_bass_kernel_spmd(trace=True)`` can capture NTFF profiles."""
    if "antenv.axon_hooks" in sys.modules:
        return
    try:
        from trn_agent_boot.trn_boot import _ntff_profile_via_ctypes

        hook = _ntff_profile_via_ctypes("/opt/axon/libaxon_pjrt.so")
        mod = types.ModuleType("antenv.axon_hooks")
        mod._hook = hook
        mod.get_axon_ntff_profile_hook = lambda: mod._hook
        mod.set_axon_ntff_profile_hook = lambda h: setattr(mod, "_hook", h)
        sys.modules["antenv.axon_hooks"] = mod
    except Exception:
        pass


_install_ntff_hook_shim()

N, D = 8192, 64
NCORES = 8
RB = N // NCORES          # rows (i) per core = 1024
NT = N // 128             # j tiles of 128 = 64
BT = RB // 128            # i tiles per core = 8
F32 = mybir.dt.float32
BF16 = mybir.dt.bfloat16
EXP = mybir.ActivationFunctionType.Exp
COPY = mybir.ActivationFunctionType.Copy
RELU = mybir.ActivationFunctionType.Relu
ADD = mybir.AluOpType.add
MUL = mybir.AluOpType.mult
MAX = mybir.AluOpType.max
AX_X = mybir.AxisListType.X
PKW = D + 131  # packed small-input width (W | b | a | ident)

# e-tiles computed on the scalar engine (relu form); the rest on DVE.
# Concentrated in early chunks so the q-fold completes mid-stream.
ACT_TILES = frozenset(
    jt for jt in range(NT) if jt % 8 in (3, 6) and jt < 48
)
# PE filler pairs after these early j-tiles (HAM warm-keeping)
FILLER_AFTER = {0: 2, 1: 2, 2: 1, 3: 1, 4: 1, 5: 1}


def build_bass() -> bass.Bass:
    nc = bacc.Bacc(None)
    xp_d = nc.declare_dram_parameter("xp", [128, NT * D], F32, isOutput=False)
    xbf_d = nc.declare_dram_parameter(
        "xbf", [128, NT * 128], BF16, isOutput=False
    )
    xbkT_d = nc.declare_dram_parameter("xbkT", [D, RB], F32, isOutput=False)
    pk_d = nc.declare_dram_parameter("pack", [128, PKW], F32, isOutput=False)
    out_d = nc.declare_dram_parameter("out", [128, BT * D], F32, isOutput=True)

    with tile.TileContext(nc) as tc:
        with (
            tc.tile_pool(name="persist", bufs=1) as persist,
            tc.tile_pool(name="small", bufs=1) as small,
            tc.tile_pool(name="work", bufs=3) as work,
            tc.tile_pool(name="epool", bufs=12) as epool,
            tc.tile_pool(name="opool", bufs=2) as opool,
            tc.tile_pool(name="psumA", bufs=2, space="PSUM") as psumA,
            tc.tile_pool(name="psumB", bufs=1, space="PSUM") as psumB,
            tc.tile_pool(name="psumG", bufs=2, space="PSUM") as psumG,
            tc.tile_pool(name="psumR", bufs=2, space="PSUM") as psumR,
        ):
            # ---- DMA issues first, top priority, sync ring ----
            pk = small.tile([128, PKW], F32)
            xbkT_sb = small.tile([D, RB], F32)
            x_flat = persist.tile([128, NT * D], F32)
            x_sb = x_flat.rearrange("p (t d) -> p t d", t=NT)
            xbf_flat = persist.tile([128, NT * 128], BF16)
            x_bf = xbf_flat.rearrange("p (t d) -> p t d", t=NT)
            CWB = 16 * 128

            def xp_dma(c):
                nc.sync.dma_start(
                    x_flat[:, 8 * c * D : 8 * (c + 1) * D],
                    xp_d[:, 8 * c * D : 8 * (c + 1) * D],
                )

            def xbf_dma(k):
                nc.sync.dma_start(
                    xbf_flat[:, k * CWB : (k + 1) * CWB],
                    xbf_d[:, k * CWB : (k + 1) * CWB],
                )

            with tc.high_priority():
                nc.sync.dma_start(pk, pk_d[:, :])
                nc.sync.dma_start(xbkT_sb, xbkT_d[:, :])
                xp_dma(0)
                xbf_dma(0)
                xp_dma(1)
                xp_dma(2)
                xbf_dma(1)
                xp_dma(3)
                xp_dma(4)
                xbf_dma(2)
                xp_dma(5)
                xp_dma(6)
                xbf_dma(3)
                xp_dma(7)

            # ---- t~0: constants, ACT exp-table preload, PE warm-up ----
            ones_row = small.tile([1, 128], F32)
            nc.vector.memset(ones_row, 1.0)
            ones_bf = small.tile([1, 512], BF16)
            nc.vector.memset(ones_bf, 1.0)
            dummy_e = small.tile([1, 1], F32)
            nc.scalar.activation(out=dummy_e, in_=ones_row[0:1, 0:1], func=EXP)
            junk_ps = psumG.tile([128, 512], F32, tag="gb", name="junk_ps")

            def filler():
                nc.tensor.matmul(
                    junk_ps,
                    lhsT=ones_bf[0:1, 0:128],
                    rhs=ones_bf,
                    start=True,
                    stop=True,
                )

            for w in range(5):
                filler()

            # ---------------- prologue + G2b (high priority) --------------
            ident = pk[:, D + 3 : D + 3 + 128]
            with tc.high_priority():
                v_ps = psumA.tile([D, 2], F32, tag="ps", name="v_ps")
                nc.tensor.matmul(
                    v_ps,
                    lhsT=pk[0:D, 0:D],
                    rhs=pk[0:D, D + 1 : D + 3],
                    start=True,
                    stop=True,
                )
                v_sb = small.tile([D, 2], F32)
                nc.vector.tensor_copy(v_sb, v_ps)

                # G2b: p2 row -> exp -> broadcast
                g2row = small.tile([1, RB], BF16)
                G2b = persist.tile([128, RB], BF16)
                for h in range(2):
                    p2r_ps = psumR.tile(
                        [1, 512], F32, tag="p2r", name="p2r_ps"
                    )
                    nc.tensor.matmul(
                        p2r_ps,
                        lhsT=v_sb[:, 1:2],
                        rhs=xbkT_sb[:, h * 512 : (h + 1) * 512],
                        start=True,
                        stop=True,
                    )
                    nc.scalar.activation(
                        out=g2row[:, h * 512 : (h + 1) * 512],
                        in_=p2r_ps,
                        func=EXP,
                        scale=-0.99,
                    )
                    gb_ps = psumG.tile([128, 512], F32, tag="gb", name="gb_ps")
                    nc.tensor.matmul(
                        gb_ps,
                        lhsT=ones_bf[0:1, 0:128],
                        rhs=g2row[:, h * 512 : (h + 1) * 512],
                        start=True,
                        stop=True,
                    )
                    if h == 0:
                        nc.scalar.copy(out=G2b[:, 0:512], in_=gb_ps)
                    else:
                        nc.vector.tensor_copy(G2b[:, 512:1024], gb_ps)

                # v1 broadcast [128, 64] for the s1 muls
                vr_ps = psumA.tile([2, D], F32, tag="ps", name="vr_ps")
                nc.tensor.transpose(vr_ps, v_sb, ident[:D, :D])
                vrow = small.tile([2, D], F32)
                nc.vector.tensor_copy(vrow, vr_ps)
                v1b_ps = psumA.tile([128, D], F32, tag="ps", name="v1b_ps")
                nc.tensor.matmul(
                    v1b_ps, lhsT=ones_row, rhs=vrow[0:1, :], start=True, stop=True
                )
                v1b = small.tile([128, D], F32)
                nc.vector.tensor_copy(v1b, v1b_ps)

                # c12 = (c1 + c2) broadcast down 128 partitions
                c_ps = psumA.tile([1, 2], F32, tag="ps", name="c_ps")
                nc.tensor.matmul(
                    c_ps,
                    lhsT=pk[0:D, D : D + 1],
                    rhs=pk[0:D, D + 1 : D + 3],
                    start=True,
                    stop=True,
                )
                c_sb = small.tile([1, 2], F32)
                nc.vector.tensor_copy(c_sb, c_ps)
                cb_ps = psumA.tile([128, 2], F32, tag="ps", name="cb_ps")
                nc.tensor.matmul(
                    cb_ps, lhsT=ones_row, rhs=c_sb, start=True, stop=True
                )
                c12 = small.tile([128, 1], F32)
                nc.vector.tensor_reduce(out=c12, in_=cb_ps, axis=AX_X, op=ADD)
                c12s = small.tile([128, 1], F32)
                nc.vector.tensor_scalar(
                    out=c12s, in0=c12, scalar1=0.01, scalar2=None, op0=MUL
                )

            v1b_b = bass.AP(
                tensor=v1b.tensor,
                offset=v1b.offset,
                ap=[v1b.ap[0], [0, 8], v1b.ap[1]],
            )

            # ---------------- s1 pipeline (DVE mul + reduce) ---------------
            s1c = small.tile([128, NT], F32)
            E1c = small.tile([128, NT], F32)
            nE1c = small.tile([128, NT], F32)   # -E1 (relu-form bias)
            E1bf = small.tile([128, NT], BF16)  # E1 (q-matmul rhs)
            F1c = small.tile([128, NT], F32)

            def s1_chunk(c):
                sl = slice(8 * c, 8 * (c + 1))
                tmp = work.tile([128, 8, D], F32, tag="tmp", name="tmp")
                nc.vector.tensor_mul(tmp, x_sb[:, sl, :], v1b_b)
                nc.vector.tensor_reduce(
                    out=s1c[:, sl], in_=tmp, axis=AX_X, op=ADD
                )
                nc.scalar.activation(
                    out=E1c[:, sl], in_=s1c[:, sl], func=EXP, bias=c12,
                    scale=1.0,
                )
                nc.scalar.activation(
                    out=F1c[:, sl], in_=s1c[:, sl], func=EXP, bias=c12s,
                    scale=0.01,
                )
                nc.vector.tensor_scalar(
                    out=nE1c[:, sl], in0=E1c[:, sl], scalar1=-1.0,
                    scalar2=None, op0=MUL,
                )
                nc.vector.tensor_copy(E1bf[:, sl], E1c[:, sl])

            for c in range(4):
                s1_chunk(c)

            # ---------------- main stream ----------------
            acc0 = psumB.tile([128, 512], F32, tag="acc0", name="acc0")
            acc1 = psumB.tile([128, 512], F32, tag="acc1", name="acc1")
            accs = [acc0, acc1]
            q_ps = psumG.tile([128, 1], F32, tag="gb", name="q_ps")
            n_act = 0
            for jt in range(NT):
                if jt % 8 == 0 and jt // 8 + 4 < 8:
                    s1_chunk(jt // 8 + 4)
                if jt == 52:
                    # fold q back mid-stream: acc[m,i] += q[m] * ones[i]
                    q_sb = small.tile([128, 1], F32)
                    nc.vector.tensor_copy(q_sb, q_ps)
                    qr_ps = psumR.tile([1, 128], F32, tag="p2r", name="qr_ps")
                    nc.tensor.transpose(qr_ps, q_sb, ident[:128, :128])
                    qrow = small.tile([1, 128], BF16)
                    nc.vector.tensor_copy(qrow, qr_ps)
                    for h in range(2):
                        nc.tensor.matmul(
                            accs[h],
                            lhsT=qrow,
                            rhs=ones_bf,
                            start=False,
                            stop=False,
                        )
                e_t = epool.tile([128, RB], BF16, tag="e", name="e_t")
                if jt in ACT_TILES:
                    # r = relu(G2b*F1[j] - E1[j]) on the scalar engine
                    nc.scalar.activation(
                        out=e_t,
                        in_=G2b,
                        func=RELU,
                        scale=F1c[:, jt : jt + 1],
                        bias=nE1c[:, jt : jt + 1],
                    )
                else:
                    # e = max(G2b*F1[j], E1[j]) on DVE
                    nc.vector.tensor_scalar(
                        out=e_t,
                        in0=G2b,
                        scalar1=F1c[:, jt : jt + 1],
                        scalar2=E1c[:, jt : jt + 1],
                        op0=MUL,
                        op1=MAX,
                    )
                for h in range(2):
                    nc.tensor.matmul(
                        accs[h],
                        lhsT=x_bf[:, jt, 0:128],
                        rhs=e_t[:, h * 512 : (h + 1) * 512],
                        start=(jt == 0),
                        stop=(jt == NT - 1),
                    )
                if jt in ACT_TILES:
                    # q += x_ext[j,:].T @ E1[j]  (the subtracted rank-1 term)
                    n_act += 1
                    nc.tensor.matmul(
                        q_ps,
                        lhsT=x_bf[:, jt, 0:128],
                        rhs=E1bf[:, jt : jt + 1],
                        start=(n_act == 1),
                        stop=(n_act == len(ACT_TILES)),
                    )
                for _ in range(FILLER_AFTER.get(jt, 0)):
                    filler()

            # ---------------- epilogue: normalize + store ----------------
            outT = small.tile([D + 1, RB], F32)
            nc.vector.tensor_copy(outT[:, 0:256], acc0[0 : D + 1, 0:256])
            nc.scalar.copy(out=outT[:, 512:768], in_=acc1[0 : D + 1, 0:256])
            nc.vector.tensor_copy(outT[:, 256:512], acc0[0 : D + 1, 256:512])
            nc.scalar.copy(out=outT[:, 768:1024], in_=acc1[0 : D + 1, 256:512])
            out_flat = small.tile([128, BT * D], F32)
            out_sb = out_flat.rearrange("p (t d) -> p t d", t=BT)
            tp_pools = [(psumA, "ps"), (psumR, "p2r"), (psumG, "gb")]
            for t in range(BT):
                pool, tag = tp_pools[t % 3]
                tp2 = pool.tile([128, D + 1], F32, tag=tag, name="tp2")
                nc.tensor.transpose(
                    tp2, outT[:, t * 128 : (t + 1) * 128], ident[: D + 1, : D + 1]
                )
                rcol = opool.tile([128, 1], F32, tag="rcol", name="rcol")
                nc.vector.reciprocal(rcol, tp2[:, D : D + 1])
                nc.scalar.activation(
                    out=out_sb[:, t, :],
                    in_=tp2[:, 0:D],
                    func=COPY,
                    scale=rcol,
                )
                if t % 2 == 1:
                    nc.sync.dma_start(
                        out_d[:, (t - 1) * D : (t + 1) * D],
                        out_flat[:, (t - 1) * D : (t + 1) * D],
                    )

    nc.finalize()
    return nc


def _execute(inputs: dict, trace: bool = False):
    x = np.ascontiguousarray(np.asarray(inputs["x"], dtype=np.float32))
    W = np.ascontiguousarray(np.asarray(inputs["W"], dtype=np.float32))
    b = np.ascontiguousarray(
        np.asarray(inputs["b"], dtype=np.float32).reshape(D, 1)
    )
    a = np.ascontiguousarray(
        np.asarray(inputs["a"], dtype=np.float32).reshape(2 * D, 1)
    )
    assert x.shape == (N, D) and W.shape == (D, D)

    nc = build_bass()
    pack0 = np.zeros((128, PKW), np.float32)
    pack0[0:D, 0:D] = W
    pack0[0:D, D] = b[:, 0]
    pack0[0:D, D + 1] = a[:D, 0]
    pack0[0:D, D + 2] = a[D:, 0]
    pack0[:, D + 3 : D + 131] = np.eye(128, dtype=np.float32)

    # host-side layout only: partition-major f32 x, bf16 x_ext, d-major
    # own-block x.T; j-tile order rotated per core (own block first)
    xp_tiles = x.reshape(NT, 128, D).transpose(1, 0, 2)  # [128, NT, D]
    xe = np.concatenate(
        [x, np.ones((N, 1), np.float32), np.zeros((N, 127 - D), np.float32)],
        axis=1,
    )
    xbf_tiles = (
        xe.reshape(NT, 128, 128).transpose(1, 0, 2).astype(ml_dtypes.bfloat16)
    )  # [128, NT, 128]
    in_maps = []
    for c in range(NCORES):
        rot = np.roll(np.arange(NT), -c * BT)  # own tiles first
        xp_c = np.ascontiguousarray(xp_tiles[:, rot, :].reshape(128, NT * D))
        xbf_c = np.ascontiguousarray(
            xbf_tiles[:, rot, :].reshape(128, NT * 128)
        )
        xbkT = np.ascontiguousarray(x[c * RB : (c + 1) * RB].T)
        in_maps.append(
            {"xp": xp_c, "xbf": xbf_c, "xbkT": xbkT, "pack": pack0}
        )
    res = run_bass_kernel_spmd(
        nc, in_maps, core_ids=list(range(NCORES)), trace=trace
    )
    # un-permute each core's output: (p, t*D+d) -> (t*128+p, d)
    outs = []
    for r in res.results:
        o = r["out"].reshape(128, BT, D).transpose(1, 0, 2).reshape(RB, D)
        outs.append(o)
    out = np.ascontiguousarray(np.concatenate(outs, axis=0))
    return out, res


def kernel(x, W, b, a):
    out, _ = _execute({"x": x, "W": W, "b": b, "a": a})
    return out


# revision 16
# speedup vs baseline: 1.0586x; 1.0380x over previous
"""GAT layer (N=8192, D=64) as a Bass/Tile kernel on 8 TRN2 NeuronCores.

Math (reference):
    h  = x @ W.T + b
    s1 = h @ a1 ; s2 = h @ a2                    # [N] each
    score[i,j] = s2[i] + s1[j]
    att = softmax_j(leaky_relu(score))
    out = att @ x

Reformulation used here:
    Fold the linear layer:  v = W.T @ [a1|a2], c_k = b.a_k
      p1 = x @ v1 ; p2 = x @ v2 ; s1 = p1 + c1 ; s2 = p2 + c2
    Softmax rows are shift invariant, so subtract p2[i] from row i:
      exp(lr(score) - p2[i]) = max( exp(sh1[j]),
                                    exp(0.01*sh1[j]) * exp(-0.99*p2[i]) )
      with sh1[j] = p1[j] + c1 + c2   (lr = leaky-relu, exp is monotone
      so exp(max(a,b)) = max(exp a, exp b))
    So with per-j-row scalars E1 = exp(sh1), F1 = exp(0.01*sh1) and a
    broadcast tile G2b[j,i] = exp(-0.99*p2[i]), the unnormalized weight
    tile (layout [j partitions, i free]) is ONE tensor_scalar op:
      e[j,i] = max( G2b[j,i] * F1[j],  E1[j] )
    The final matmul (with a ones-column appended to x to get the
    softmax denominator for free) accumulates over j in PSUM:
      outT[0:64, i] += x_ext[j,:].T @ e[j, i] ; Z[i] = outT[64, i]

Sharding: each core owns N/8 = 1024 query rows i (full x is only 2MB and
is replicated to every core), no collectives needed. Inputs are shipped
pre-permuted to partition-major layout (p, t, d) so every DMA is a flat
contiguous 2D transfer. Engine roles: DVE runs the 64 fused
mult+max tensor_scalar tiles (the critical stream, ~35us), PE runs the
128 accumulating matmuls (~32us, overlapped), ACT does exps and
PSUM->SBUF copies, gpsimd does the s1 elementwise multiplies.
"""

import sys
import types

import ml_dtypes
import numpy as np

import concourse.bacc as bacc
import concourse.bass as bass
import concourse.mybir as mybir
import concourse.tile as tile
from concourse.bass_utils import run_bass_kernel_spmd


def _install_ntff_hook_shim():
    """The agent image's ``antenv`` lacks ``axon_hooks``; provide it so
    ``run_bass_kernel_spmd(trace=True)`` can capture NTFF profiles. The
    actual hook implementation ships with the axon boot package."""
    if "antenv.axon_hooks" in sys.modules:
        return
    try:
        from trn_agent_boot.trn_boot import _ntff_profile_via_ctypes

        hook = _ntff_profile_via_ctypes("/opt/axon/libaxon_pjrt.so")
        mod = types.ModuleType("antenv.axon_hooks")
        mod._hook = hook
        mod.get_axon_ntff_profile_hook = lambda: mod._hook
        mod.set_axon_ntff_profile_hook = lambda h: setattr(mod, "_hook", h)
        sys.modules["antenv.axon_hooks"] = mod
    except Exception:
        pass


_install_ntff_hook_shim()

N, D = 8192, 64
NCORES = 8
RB = N // NCORES          # rows (i) per core = 1024
NT = N // 128             # j tiles of 128 = 64
BT = RB // 128            # i tiles per core = 8
F32 = mybir.dt.float32
BF16 = mybir.dt.bfloat16
EXP = mybir.ActivationFunctionType.Exp
ADD = mybir.AluOpType.add
MUL = mybir.AluOpType.mult
MAX = mybir.AluOpType.max
AX_X = mybir.AxisListType.X
PKW = D + 131  # packed small-input width (W | b | a | ident)


def build_bass() -> bass.Bass:
    nc = bacc.Bacc(None)
    # partition-major (p, t, d) layouts, prepared on the host
    xp_d = nc.declare_dram_parameter("xp", [128, NT * D], F32, isOutput=False)
    xbf_d = nc.declare_dram_parameter(
        "xbf", [128, NT * 128], BF16, isOutput=False
    )
    pk_d = nc.declare_dram_parameter("pack", [128, PKW], F32, isOutput=False)
    xbkT_d = nc.declare_dram_parameter("xbkT", [D, RB], F32, isOutput=False)
    out_d = nc.declare_dram_parameter("out", [128, BT * D], F32, isOutput=True)

    with tile.TileContext(nc) as tc:
        with (
            tc.tile_pool(name="persist", bufs=1) as persist,
            tc.tile_pool(name="small", bufs=1) as small,
            tc.tile_pool(name="work", bufs=3) as work,
            tc.tile_pool(name="epool", bufs=12) as epool,
            tc.tile_pool(name="opool", bufs=2) as opool,
            tc.tile_pool(name="psumA", bufs=3, space="PSUM") as psumA,
            tc.tile_pool(name="psumB", bufs=1, space="PSUM") as psumB,
        ):
            # ------- all small inputs arrive in ONE packed DMA -------
            pk = small.tile([128, PKW], F32)
            nc.sync.dma_start(pk, pk_d[:, :])
            W_sb = pk[0:D, 0:D]
            b_sb = pk[0:D, D : D + 1]
            a_sb = pk[0:D, D + 1 : D + 3]
            ident = pk[:, D + 3 : D + 3 + 128]
            ones_row = small.tile([1, 128], F32)
            nc.vector.memset(ones_row, 1.0)
            ones_bf = small.tile([1, 128], BF16)
            nc.vector.memset(ones_bf, 1.0)

            # ------- x loads: flat contiguous 2D chunks on the SP queue -------
            # (each dma_start costs ~2us of descriptor generation on the
            # issuing engine, so few, large, contiguous transfers win)
            x_flat = persist.tile([128, NT * D], F32)
            x_sb = x_flat.rearrange("p (t d) -> p t d", t=NT)
            xchunks = [(0, 8), (8, 8), (16, 16), (32, 16), (48, 16)]
            t0w, n0w = xchunks[0]
            nc.sync.dma_start(
                x_flat[:, t0w * D : (t0w + n0w) * D],
                xp_d[:, t0w * D : (t0w + n0w) * D],
            )
            xbkT_sb = small.tile([D, RB], F32)
            nc.sync.dma_start(xbkT_sb, xbkT_d[:, :])
            for tw, nw in xchunks[1:]:
                nc.sync.dma_start(
                    x_flat[:, tw * D : (tw + nw) * D],
                    xp_d[:, tw * D : (tw + nw) * D],
                )
            # bf16 x (with ones column folded in)
            xbf_flat = persist.tile([128, NT * 128], BF16)
            x_bf = xbf_flat.rearrange("p (t d) -> p t d", t=NT)
            CWB = 32 * 128
            for c in range(2):
                nc.sync.dma_start(
                    xbf_flat[:, c * CWB : (c + 1) * CWB],
                    xbf_d[:, c * CWB : (c + 1) * CWB],
                )

            # ---------------- tiny projections on PE ----------------
            # v = W.T @ [a1|a2]  [64,2] ;  c = [b.a1, b.a2]  [1,2]
            v_ps = psumA.tile([D, 2], F32, tag="ps", name="v_ps")
            nc.tensor.matmul(v_ps, lhsT=W_sb, rhs=a_sb, start=True, stop=True)
            v_sb = small.tile([D, 2], F32)
            nc.scalar.copy(out=v_sb, in_=v_ps)

            c_ps = psumA.tile([1, 2], F32, tag="ps", name="c_ps")
            nc.tensor.matmul(c_ps, lhsT=b_sb, rhs=a_sb, start=True, stop=True)
            c_sb = small.tile([1, 2], F32)
            nc.scalar.copy(out=c_sb, in_=c_ps)

            # c12 = (c1 + c2) broadcast down 128 partitions
            cb_ps = psumA.tile([128, 2], F32, tag="ps", name="cb_ps")
            nc.tensor.matmul(cb_ps, lhsT=ones_row, rhs=c_sb, start=True, stop=True)
            c12 = small.tile([128, 1], F32)
            nc.vector.tensor_reduce(out=c12, in_=cb_ps, axis=AX_X, op=ADD)
            c12s = small.tile([128, 1], F32)
            nc.vector.tensor_scalar(
                out=c12s, in0=c12, scalar1=0.01, scalar2=None, op0=MUL
            )

            # v1 / v2 rows (via PE transpose) and partition broadcasts
            v1r_ps = psumA.tile([1, D], F32, tag="ps", name="v1r_ps")
            nc.tensor.transpose(v1r_ps, v_sb[:, 0:1], ident[:D, :D])
            v1row = small.tile([1, D], F32)
            nc.scalar.copy(out=v1row, in_=v1r_ps)
            v1b_ps = psumA.tile([128, D], F32, tag="ps", name="v1b_ps")
            nc.tensor.matmul(
                v1b_ps, lhsT=ones_row, rhs=v1row, start=True, stop=True
            )
            v1b = small.tile([128, D], F32)
            nc.scalar.copy(out=v1b, in_=v1b_ps)
            # ---------------- p2 for this block -> G2b ----------------
            # p2row = v2.T @ xblk.T  (xblk.T shipped from host)
            G2b = persist.tile([128, RB], BF16)
            for h in range(2):
                p2r_ps = psumA.tile([1, 512], F32, tag="ps", name="p2r_ps")
                nc.tensor.matmul(
                    p2r_ps,
                    lhsT=v_sb[:, 1:2],
                    rhs=xbkT_sb[:, h * 512 : (h + 1) * 512],
                    start=True,
                    stop=True,
                )
                g2row = small.tile([1, 512], BF16, tag="g2row", name="g2row")
                nc.scalar.activation(out=g2row, in_=p2r_ps, func=EXP, scale=-0.99)
                gb_ps = psumA.tile([128, 512], F32, tag="ps", name="gb_ps")
                nc.tensor.matmul(
                    gb_ps, lhsT=ones_bf, rhs=g2row, start=True, stop=True
                )
                nc.scalar.copy(
                    out=G2b[:, h * 512 : (h + 1) * 512], in_=gb_ps
                )

            # ---------------- s1 columns + exps ----------------
            # s1c[p, jt] = sum_d x[jt*128+p, d] * v1[d]
            s1c = small.tile([128, NT], F32)
            E1c = small.tile([128, NT], F32)
            F1c = small.tile([128, NT], F32)
            v1b_b = bass.AP(
                tensor=v1b.tensor,
                offset=v1b.offset,
                ap=[v1b.ap[0], [0, 8], v1b.ap[1]],
            )
            acc0 = psumB.tile([128, 512], F32, tag="acc0", name="acc0")
            acc1 = psumB.tile([128, 512], F32, tag="acc1", name="acc1")
            accs = [acc0, acc1]
            # interleaved: per 8-tile chunk, s1 projections then main tiles
            # (keeps the DVE from idle-waiting on gpsimd-paced reduces)
            for c in range(8):
                tmp = work.tile([128, 8, D], F32, tag="tmp", name="tmp")
                nc.gpsimd.tensor_mul(
                    tmp, x_sb[:, 8 * c : 8 * (c + 1), :], v1b_b
                )
                nc.vector.tensor_reduce(
                    out=s1c[:, 8 * c : 8 * (c + 1)], in_=tmp, axis=AX_X, op=ADD
                )
                nc.scalar.activation(
                    out=E1c[:, 8 * c : 8 * (c + 1)],
                    in_=s1c[:, 8 * c : 8 * (c + 1)],
                    func=EXP,
                    bias=c12,
                    scale=1.0,
                )
                nc.scalar.activation(
                    out=F1c[:, 8 * c : 8 * (c + 1)],
                    in_=s1c[:, 8 * c : 8 * (c + 1)],
                    func=EXP,
                    bias=c12s,
                    scale=0.01,
                )
                for jt in range(8 * c, 8 * (c + 1)):
                    e_t = epool.tile([128, RB], BF16, tag="e", name="e_t")
                    # e[j,i] = max(G2b[j,i] * F1[j], E1[j])
                    nc.vector.tensor_scalar(
                        out=e_t,
                        in0=G2b,
                        scalar1=F1c[:, jt : jt + 1],
                        scalar2=E1c[:, jt : jt + 1],
                        op0=MUL,
                        op1=MAX,
                    )
                    for h in range(2):
                        nc.tensor.matmul(
                            accs[h],
                            lhsT=x_bf[:, jt, 0:128],
                            rhs=e_t[:, h * 512 : (h + 1) * 512],
                            start=(jt == 0),
                            stop=(jt == NT - 1),
                        )

            # ---------------- epilogue: normalize + store ----------------
            outT = small.tile([D + 1, RB], F32)
            for h in range(2):
                nc.scalar.copy(
                    out=outT[:, h * 512 : (h + 1) * 512],
                    in_=accs[h][0 : D + 1, :],
                )
            out_flat = small.tile([128, BT * D], F32)
            out_sb = out_flat.rearrange("p (t d) -> p t d", t=BT)
            for t in range(BT):
                tp2 = psumA.tile([128, D + 1], F32, tag="ps", name="tp2")
                nc.tensor.transpose(
                    tp2, outT[:, t * 128 : (t + 1) * 128], ident[: D + 1, : D + 1]
                )
                rcol = opool.tile([128, 1], F32, tag="rcol", name="rcol")
                nc.vector.reciprocal(rcol, tp2[:, D : D + 1])
                nc.vector.tensor_scalar(
                    out=out_sb[:, t, :],
                    in0=tp2[:, 0:D],
                    scalar1=rcol,
                    scalar2=None,
                    op0=MUL,
                )
            nc.sync.dma_start(out_d[:, :], out_flat)

    nc.finalize()
    return nc


def _execute(inputs: dict, trace: bool = False):
    x = np.ascontiguousarray(np.asarray(inputs["x"], dtype=np.float32))
    W = np.ascontiguousarray(np.asarray(inputs["W"], dtype=np.float32))
    b = np.ascontiguousarray(
        np.asarray(inputs["b"], dtype=np.float32).reshape(D, 1)
    )
    a = np.ascontiguousarray(
        np.asarray(inputs["a"], dtype=np.float32).reshape(2 * D, 1)
    )
    assert x.shape == (N, D) and W.shape == (D, D)

    # partition-major permutations: (t*128+p, d) -> (p, t*D+d)
    xp = np.ascontiguousarray(
        x.reshape(NT, 128, D).transpose(1, 0, 2).reshape(128, NT * D)
    )
    xe = np.concatenate(
        [x, np.ones((N, 1), np.float32), np.zeros((N, 127 - D), np.float32)],
        axis=1,
    )
    xbf = np.ascontiguousarray(
        xe.reshape(NT, 128, 128)
        .transpose(1, 0, 2)
        .reshape(128, NT * 128)
        .astype(ml_dtypes.bfloat16)
    )
    nc = build_bass()
    pack0 = np.zeros((128, PKW), np.float32)
    pack0[0:D, 0:D] = W
    pack0[0:D, D] = b[:, 0]
    pack0[0:D, D + 1] = a[:D, 0]
    pack0[0:D, D + 2] = a[D:, 0]
    pack0[:, D + 3 : D + 131] = np.eye(128, dtype=np.float32)
    in_maps = []
    for c in range(NCORES):
        xbkT = np.ascontiguousarray(x[c * RB : (c + 1) * RB].T)
        in_maps.append({"xp": xp, "xbf": xbf, "pack": pack0, "xbkT": xbkT})
    res = run_bass_kernel_spmd(
        nc, in_maps, core_ids=list(range(NCORES)), trace=trace
    )
    # un-permute each core's output: (p, t*D+d) -> (t*128+p, d)
    outs = []
    for r in res.results:
        o = r["out"].reshape(128, BT, D).transpose(1, 0, 2).reshape(RB, D)
        outs.append(o)
    out = np.ascontiguousarray(np.concatenate(outs, axis=0))
    return out, res


def kernel(x, W, b, a):
    out, _ = _execute({"x": x, "W": W, "b": b, "a": a})
    return out

